# revision 12
# baseline (speedup 1.0000x reference)
"""CenterLoss kernel for Trainium2 (8 NeuronCores, Bass/Tile).

Computes, for features [B, D], labels [B], center_var [V, D]:
    centers_batch = center_var[labels]
    loss          = mean((features - centers_batch)^2)
    new_centers   = center_var.at[labels].add(0.05 * (features - centers_batch))

Sharding: center_var is row-sharded across 8 cores (12500 rows each).
Each (feature, label) pair is routed on host to the core that owns its
label (the expert-parallel all-to-all), sorted by label, and bin-packed
into 128-slot tiles such that no label's duplicate run crosses a tile
boundary.  On device, each 128-pair tile gathers its center rows with an
indirect DMA, merges duplicate labels with an is_equal selection-matrix
matmul (PE accumulates duplicate updates; colliding scatter writes then
carry identical values), and scatters the updated rows back.  The bulk
shard copy is split into chunks; each scatter only waits for the copy
chunks covering its (sorted, hence contiguous) row band, so scatters
stream while the copy is still in flight.  The scalar loss is reduced
per core and summed on host.
"""

import numpy as np

NUM_CLASSES = 100000
EMBED_DIM = 512
BATCH = 16384
N_CORES = 8
SHARD = NUM_CLASSES // N_CORES  # 12500
P = 128
N_CHUNKS = 40
CHUNK = -(-SHARD // N_CHUNKS)  # 313; last chunk is smaller
_CHUNK_BOUNDS = [min(i * CHUNK, SHARD) for i in range(N_CHUNKS + 1)]

_PROGRAM_CACHE = {}


def _route(labels):
    """Route pairs to shard-owning cores; bin-pack into 128-slot tiles so a
    duplicate-label run never crosses a tile boundary.

    Returns (slot_feat_idx [N_CORES, T*P] int64 with -1 padding,
             slot_local    [N_CORES, T*P] int32 with SHARD padding, T).
    """
    labels = np.asarray(labels).astype(np.int64).reshape(-1)
    owner = labels // SHARD
    local = (labels % SHARD).astype(np.int32)

    per_core = []
    for c in range(N_CORES):
        sel = np.nonzero(owner == c)[0]
        loc = local[sel]
        order = np.argsort(loc, kind="stable")
        sel, loc = sel[order], loc[order]
        n = len(loc)
        if n == 0:
            per_core.append((np.empty(0, np.int64), np.empty(0, np.int32)))
            continue
        # run boundaries of equal labels
        change = np.nonzero(np.diff(loc))[0] + 1
        run_starts = np.concatenate(([0], change))
        run_lens = np.diff(np.concatenate((run_starts, [n])))
        if run_lens.max() > P:
            raise ValueError("a single label occurs more than 128 times")
        # greedy bin-pack: slot position for each run
        run_slots = np.empty(len(run_starts), np.int64)
        cur = 0
        for r, L in enumerate(run_lens):
            used = cur % P
            if used + L > P:
                cur += P - used
            run_slots[r] = cur
            cur += L
        pair_run = np.repeat(np.arange(len(run_starts)), run_lens)
        within = np.arange(n) - run_starts[pair_run]
        slots = run_slots[pair_run] + within
        total = int(cur)
        feat_idx = np.full(total, -1, np.int64)
        loc_out = np.full(total, SHARD, np.int32)
        feat_idx[slots] = sel
        loc_out[slots] = loc
        per_core.append((feat_idx, loc_out))

    max_slots = max(max(len(a) for a, _ in per_core), 1)
    T = -(-max_slots // P)
    n_slots = T * P
    slot_feat_idx = np.full((N_CORES, n_slots), -1, np.int64)
    slot_local = np.full((N_CORES, n_slots), SHARD, np.int32)
    for c, (fi, lo) in enumerate(per_core):
        slot_feat_idx[c, : len(fi)] = fi
        slot_local[c, : len(lo)] = lo
    return slot_feat_idx, slot_local, T


def _tile_chunk_deps(slot_local, T):
    """Per tile, the copy-chunk range [clo, chi] covering the union (over
    cores) of the tile's real local rows.  All-pad tiles get (0, -1)."""
    deps = []
    for t in range(T):
        rows = slot_local[:, t * P : (t + 1) * P]
        real = rows[rows < SHARD]
        if real.size == 0:
            deps.append((0, -1))
        else:
            deps.append((int(real.min()) // CHUNK, int(real.max()) // CHUNK))
    return tuple(deps)


def _build_program(T, tile_deps):
    """Build + compile the SPMD Bass program for T pair-tiles per core."""
    import concourse.bass as bass
    import concourse.tile as tile
    from concourse import bacc, mybir
    from concourse.masks import make_identity
    from concourse.tile_rust import add_dep_helper

    f32 = mybir.dt.float32
    i32 = mybir.dt.int32

    nc = bacc.Bacc("TRN2", target_bir_lowering=False, debug=False,
                   num_devices=N_CORES)

    shard_ap = nc.dram_tensor("shard", [SHARD + 1, EMBED_DIM], f32,
                              kind="ExternalInput").ap()
    feat_ap = nc.dram_tensor("feat", [P, T * EMBED_DIM], f32,
                             kind="ExternalInput").ap()
    idx_ap = nc.dram_tensor("idx", [P, T], i32, kind="ExternalInput").ap()
    out_ap = nc.dram_tensor("out", [SHARD + 1, EMBED_DIM], f32,
                            kind="ExternalOutput").ap()
    loss_ap = nc.dram_tensor("loss", [1, 1], f32, kind="ExternalOutput").ap()

    with tile.TileContext(nc) as tc:
        with (
            tc.tile_pool(name="const", bufs=1) as const_pool,
            tc.tile_pool(name="stage", bufs=1) as stage_pool,
            tc.tile_pool(name="work", bufs=4) as work_pool,
            tc.tile_pool(name="newp", bufs=max(T, 1)) as new_pool,
            tc.tile_pool(name="small", bufs=4) as small_pool,
            tc.tile_pool(name="psum", bufs=2, space="PSUM") as psum_pool,
            tc.tile_pool(name="psumT", bufs=2, space="PSUM") as psumT_pool,
        ):
            # stage routed features / indices FIRST: HWDGE queues are FIFO
            # per issuing engine, so these must not sit behind the bulk copy
            idx_sb = stage_pool.tile([P, T], i32)
            nc.sync.dma_start(idx_sb[:], idx_ap[:])
            feat_sb = stage_pool.tile([P, T * EMBED_DIM], f32)
            nc.scalar.dma_start(feat_sb[:], feat_ap[:])

            # bulk copy of the shard rows input -> output in chunks,
            # alternating between the two HWDGE queues
            copy_insts = []
            for ci in range(N_CHUNKS):
                lo, hi = _CHUNK_BOUNDS[ci], _CHUNK_BOUNDS[ci + 1]
                eng = nc.sync if ci % 2 == 0 else nc.scalar
                copy_insts.append(
                    eng.dma_start(out=out_ap[lo:hi], in_=shard_ap[lo:hi]))

            identity = const_pool.tile([P, P], f32)
            make_identity(nc, identity[:])
            loss_cols = stage_pool.tile([P, T], f32)

            for t in range(T):
                idx_t = idx_sb[:, t : t + 1]
                feat_t = feat_sb[:, t * EMBED_DIM : (t + 1) * EMBED_DIM]

                # gather original center rows for this tile's pairs
                c_t = work_pool.tile([P, EMBED_DIM], f32, tag="c")
                nc.gpsimd.indirect_dma_start(
                    out=c_t[:],
                    out_offset=None,
                    in_=shard_ap[:],
                    in_offset=bass.IndirectOffsetOnAxis(ap=idx_t, axis=0),
                )

                # selection matrix sel[i,j] = (label_i == label_j) within tile
                idxf = small_pool.tile([P, 1], f32, tag="idxf")
                nc.vector.tensor_copy(idxf[:], idx_t)
                pT = psumT_pool.tile([P, P], f32, space="PSUM", tag="pT")
                nc.tensor.transpose(out=pT[:], in_=idxf[:].to_broadcast([P, P]),
                                    identity=identity[:])
                sel = small_pool.tile([P, P], f32, tag="sel")
                nc.vector.tensor_tensor(
                    out=sel[:], in0=idxf[:].to_broadcast([P, P]), in1=pT[:],
                    op=mybir.AluOpType.is_equal)

                # e = f - c ; loss column = sum(e^2) fused into ACT square
                e_t = work_pool.tile([P, EMBED_DIM], f32, tag="e")
                nc.vector.tensor_tensor(
                    out=e_t[:], in0=feat_t, in1=c_t[:],
                    op=mybir.AluOpType.subtract)
                esq = work_pool.tile([P, EMBED_DIM], f32, tag="esq")
                nc.scalar.activation(
                    out=esq[:], in_=e_t[:],
                    func=mybir.ActivationFunctionType.Square,
                    accum_out=loss_cols[:, t : t + 1])

                # accumulate duplicate-label updates: acc = sel @ e
                acc = psum_pool.tile([P, EMBED_DIM], f32, space="PSUM", tag="acc")
                nc.tensor.matmul(out=acc[:], lhsT=sel[:], rhs=e_t[:],
                                 start=True, stop=True)

                # new rows = c + 0.05 * acc ; scatter back once the copy
                # chunks covering this tile's row band have landed
                new_t = new_pool.tile([P, EMBED_DIM], f32, tag="new")
                nc.vector.scalar_tensor_tensor(
                    out=new_t[:], in0=acc[:], scalar=0.05, in1=c_t[:],
                    op0=mybir.AluOpType.mult, op1=mybir.AluOpType.add)
                sc = nc.gpsimd.indirect_dma_start(
                    out=out_ap[:],
                    out_offset=bass.IndirectOffsetOnAxis(ap=idx_t, axis=0),
                    in_=new_t[:],
                    in_offset=None,
                    bounds_check=SHARD - 1,
                    oob_is_err=False,
                )
                clo, chi = tile_deps[t]
                for ci in range(clo, chi + 1):
                    add_dep_helper(sc.ins, copy_insts[ci].ins, sync=True,
                                   reason="scatter after its copy chunks")

            # reduce loss: [P, T] -> [P, 1] -> scalar via matmul with ones
            lsum = small_pool.tile([P, 1], f32, tag="lsum")
            nc.vector.tensor_reduce(out=lsum[:], in_=loss_cols[:, :T],
                                    axis=mybir.AxisListType.X,
                                    op=mybir.AluOpType.add)
            ones = const_pool.tile([P, 1], f32)
            nc.vector.memset(ones[:], 1.0)
            lscalar = psum_pool.tile([1, 1], f32, space="PSUM", tag="lscalar")
            nc.tensor.matmul(out=lscalar[:], lhsT=lsum[:], rhs=ones[:],
                             start=True, stop=True)
            loss_sb = small_pool.tile([1, 1], f32, tag="loss_sb")
            nc.vector.tensor_copy(out=loss_sb[:], in_=lscalar[:])
            nc.sync.dma_start(loss_ap[:], loss_sb[:])

    nc.compile()
    return nc


def _make_in_maps(features, center_var, slot_feat_idx, slot_local, T):
    feat_padded = np.concatenate(
        [features, np.zeros((1, EMBED_DIM), np.float32)], axis=0)
    in_maps = []
    for c in range(N_CORES):
        shard_h = np.concatenate(
            [center_var[c * SHARD : (c + 1) * SHARD],
             np.zeros((1, EMBED_DIM), np.float32)], axis=0)
        # slot s = t*P + p  ->  SBUF layout [p, t]
        fi = slot_feat_idx[c].reshape(T, P)
        feat_h = np.ascontiguousarray(
            feat_padded[fi].transpose(1, 0, 2).reshape(P, T * EMBED_DIM))
        idx_h = np.ascontiguousarray(slot_local[c].reshape(T, P).T)
        in_maps.append({"shard": shard_h, "feat": feat_h, "idx": idx_h})
    return in_maps


def kernel(features, labels, center_var):
    from concourse.bass_utils import run_bass_kernel_spmd

    features = np.ascontiguousarray(np.asarray(features), dtype=np.float32)
    center_var = np.ascontiguousarray(np.asarray(center_var), dtype=np.float32)

    slot_feat_idx, slot_local, T = _route(labels)
    tile_deps = _tile_chunk_deps(slot_local, T)

    key = (T, tile_deps)
    if key not in _PROGRAM_CACHE:
        _PROGRAM_CACHE[key] = _build_program(T, tile_deps)
    nc = _PROGRAM_CACHE[key]

    in_maps = _make_in_maps(features, center_var, slot_feat_idx, slot_local, T)
    res = run_bass_kernel_spmd(nc, in_maps, list(range(N_CORES)))

    new_centers = np.concatenate(
        [res.results[c]["out"][:SHARD] for c in range(N_CORES)], axis=0)
    loss_sum = sum(float(res.results[c]["loss"][0, 0]) for c in range(N_CORES))
    loss = np.float32(loss_sum / (BATCH * EMBED_DIM))
    return loss, new_centers


# revision 13
# speedup vs baseline: 2.2501x; 2.2501x over previous
"""CenterLoss kernel for Trainium2 (8 NeuronCores, Bass/Tile).

Computes, for features [B, D], labels [B], center_var [V, D]:
    centers_batch = center_var[labels]
    loss          = mean((features - centers_batch)^2)
    new_centers   = center_var.at[labels].add(0.05 * (features - centers_batch))

Sharding: center_var is row-sharded across 8 cores (12500 rows each).
Each (feature, label) pair is routed on host to the core that owns its
label (the expert-parallel all-to-all), sorted by label, and bin-packed
into 128-slot tiles such that no label's duplicate run crosses a tile
boundary.  On device, each 128-pair tile gathers its center rows with an
indirect DMA, merges duplicate labels with an is_equal selection-matrix
matmul (PE accumulates duplicate updates; colliding scatter writes then
carry identical values), and scatters the updated rows back.  The bulk
shard copy is split into chunks; each scatter only waits for the copy
chunks covering its (sorted, hence contiguous) row band, so scatters
stream while the copy is still in flight.  The scalar loss is reduced
per core and summed on host.
"""

import numpy as np

NUM_CLASSES = 100000
EMBED_DIM = 512
BATCH = 16384
N_CORES = 8
SHARD = NUM_CLASSES // N_CORES  # 12500
P = 128
N_CHUNKS = 40
CHUNK = -(-SHARD // N_CHUNKS)  # 313; last chunk is smaller
_CHUNK_BOUNDS = [min(i * CHUNK, SHARD) for i in range(N_CHUNKS + 1)]

_PROGRAM_CACHE = {}


def _route(labels):
    """Route pairs to shard-owning cores; bin-pack into 128-slot tiles so a
    duplicate-label run never crosses a tile boundary.

    Returns (slot_feat_idx [N_CORES, T*P] int64 with -1 padding,
             slot_local    [N_CORES, T*P] int32 with SHARD padding, T).
    """
    labels = np.asarray(labels).astype(np.int64).reshape(-1)
    owner = labels // SHARD
    local = (labels % SHARD).astype(np.int32)

    per_core = []
    for c in range(N_CORES):
        sel = np.nonzero(owner == c)[0]
        loc = local[sel]
        order = np.argsort(loc, kind="stable")
        sel, loc = sel[order], loc[order]
        n = len(loc)
        if n == 0:
            per_core.append((np.empty(0, np.int64), np.empty(0, np.int32)))
            continue
        # run boundaries of equal labels
        change = np.nonzero(np.diff(loc))[0] + 1
        run_starts = np.concatenate(([0], change))
        run_lens = np.diff(np.concatenate((run_starts, [n])))
        if run_lens.max() > P:
            raise ValueError("a single label occurs more than 128 times")
        # greedy bin-pack: slot position for each run
        run_slots = np.empty(len(run_starts), np.int64)
        cur = 0
        for r, L in enumerate(run_lens):
            used = cur % P
            if used + L > P:
                cur += P - used
            run_slots[r] = cur
            cur += L
        pair_run = np.repeat(np.arange(len(run_starts)), run_lens)
        within = np.arange(n) - run_starts[pair_run]
        slots = run_slots[pair_run] + within
        total = int(cur)
        feat_idx = np.full(total, -1, np.int64)
        loc_out = np.full(total, SHARD, np.int32)
        feat_idx[slots] = sel
        loc_out[slots] = loc
        per_core.append((feat_idx, loc_out))

    max_slots = max(max(len(a) for a, _ in per_core), 1)
    T = -(-max_slots // P)
    n_slots = T * P
    slot_feat_idx = np.full((N_CORES, n_slots), -1, np.int64)
    slot_local = np.full((N_CORES, n_slots), SHARD, np.int32)
    for c, (fi, lo) in enumerate(per_core):
        slot_feat_idx[c, : len(fi)] = fi
        slot_local[c, : len(lo)] = lo
    return slot_feat_idx, slot_local, T


def _tile_chunk_deps(slot_local, T):
    """Per tile, the copy-chunk range [clo, chi] covering the union (over
    cores) of the tile's real local rows.  All-pad tiles get (0, -1)."""
    deps = []
    for t in range(T):
        rows = slot_local[:, t * P : (t + 1) * P]
        real = rows[rows < SHARD]
        if real.size == 0:
            deps.append((0, -1))
        else:
            deps.append((int(real.min()) // CHUNK, int(real.max()) // CHUNK))
    return tuple(deps)


def _build_program(T, tile_deps):
    """Build + compile the SPMD Bass program for T pair-tiles per core."""
    import concourse.bass as bass
    import concourse.tile as tile
    from concourse import bacc, mybir
    from concourse.masks import make_identity
    from concourse.tile_rust import add_dep_helper

    f32 = mybir.dt.float32
    i32 = mybir.dt.int32

    nc = bacc.Bacc("TRN2", target_bir_lowering=False, debug=False,
                   num_devices=N_CORES)

    shard_ap = nc.dram_tensor("shard", [SHARD + 1, EMBED_DIM], f32,
                              kind="ExternalInput").ap()
    feat_ap = nc.dram_tensor("feat", [P, T * EMBED_DIM], f32,
                             kind="ExternalInput").ap()
    idx_ap = nc.dram_tensor("idx", [P, T], i32, kind="ExternalInput").ap()
    out_ap = nc.dram_tensor("out", [SHARD + 1, EMBED_DIM], f32,
                            kind="ExternalOutput").ap()
    loss_ap = nc.dram_tensor("loss", [1, 1], f32, kind="ExternalOutput").ap()

    with tile.TileContext(nc) as tc:
        with (
            tc.tile_pool(name="const", bufs=1) as const_pool,
            tc.tile_pool(name="stage", bufs=1) as stage_pool,
            tc.tile_pool(name="work", bufs=4) as work_pool,
            tc.tile_pool(name="newp", bufs=max(T, 1)) as new_pool,
            tc.tile_pool(name="small", bufs=4) as small_pool,
            tc.tile_pool(name="psum", bufs=2, space="PSUM") as psum_pool,
            tc.tile_pool(name="psumT", bufs=2, space="PSUM") as psumT_pool,
        ):
            # stage routed features / indices FIRST: HWDGE queues are FIFO
            # per issuing engine, so these must not sit behind the bulk copy
            idx_sb = stage_pool.tile([P, T], i32)
            nc.sync.dma_start(idx_sb[:], idx_ap[:])
            feat_sb = stage_pool.tile([P, T * EMBED_DIM], f32)
            nc.scalar.dma_start(feat_sb[:], feat_ap[:])

            # bulk copy of the shard rows input -> output in chunks,
            # alternating between the two HWDGE queues
            copy_insts = []
            for ci in range(N_CHUNKS):
                lo, hi = _CHUNK_BOUNDS[ci], _CHUNK_BOUNDS[ci + 1]
                eng = nc.sync if ci % 2 == 0 else nc.scalar
                copy_insts.append(
                    eng.dma_start(out=out_ap[lo:hi], in_=shard_ap[lo:hi]))

            identity = const_pool.tile([P, P], f32)
            make_identity(nc, identity[:])
            loss_cols = stage_pool.tile([P, T], f32)

            for t in range(T):
                idx_t = idx_sb[:, t : t + 1]
                feat_t = feat_sb[:, t * EMBED_DIM : (t + 1) * EMBED_DIM]

                # gather original center rows for this tile's pairs
                c_t = work_pool.tile([P, EMBED_DIM], f32, tag="c")
                nc.gpsimd.indirect_dma_start(
                    out=c_t[:],
                    out_offset=None,
                    in_=shard_ap[:],
                    in_offset=bass.IndirectOffsetOnAxis(ap=idx_t, axis=0),
                )

                # selection matrix sel[i,j] = (label_i == label_j) within tile
                idxf = small_pool.tile([P, 1], f32, tag="idxf")
                nc.vector.tensor_copy(idxf[:], idx_t)
                pT = psumT_pool.tile([P, P], f32, space="PSUM", tag="pT")
                nc.tensor.transpose(out=pT[:], in_=idxf[:].to_broadcast([P, P]),
                                    identity=identity[:])
                sel = small_pool.tile([P, P], f32, tag="sel")
                nc.vector.tensor_tensor(
                    out=sel[:], in0=idxf[:].to_broadcast([P, P]), in1=pT[:],
                    op=mybir.AluOpType.is_equal)

                # e = f - c ; loss column = sum(e^2) fused into ACT square
                e_t = work_pool.tile([P, EMBED_DIM], f32, tag="e")
                nc.vector.tensor_tensor(
                    out=e_t[:], in0=feat_t, in1=c_t[:],
                    op=mybir.AluOpType.subtract)
                esq = work_pool.tile([P, EMBED_DIM], f32, tag="esq")
                nc.scalar.activation(
                    out=esq[:], in_=e_t[:],
                    func=mybir.ActivationFunctionType.Square,
                    accum_out=loss_cols[:, t : t + 1])

                # accumulate duplicate-label updates: acc = sel @ e
                acc = psum_pool.tile([P, EMBED_DIM], f32, space="PSUM", tag="acc")
                nc.tensor.matmul(out=acc[:], lhsT=sel[:], rhs=e_t[:],
                                 start=True, stop=True)

                # new rows = c + 0.05 * acc ; scatter back once the copy
                # chunks covering this tile's row band have landed
                new_t = new_pool.tile([P, EMBED_DIM], f32, tag="new")
                nc.vector.scalar_tensor_tensor(
                    out=new_t[:], in0=acc[:], scalar=0.05, in1=c_t[:],
                    op0=mybir.AluOpType.mult, op1=mybir.AluOpType.add)
                sc = nc.gpsimd.indirect_dma_start(
                    out=out_ap[:],
                    out_offset=bass.IndirectOffsetOnAxis(ap=idx_t, axis=0),
                    in_=new_t[:],
                    in_offset=None,
                    bounds_check=SHARD - 1,
                    oob_is_err=False,
                )
                clo, chi = tile_deps[t]
                for ci in range(clo, chi + 1):
                    add_dep_helper(sc.ins, copy_insts[ci].ins, sync=True,
                                   reason="scatter after its copy chunks")

            # reduce loss: [P, T] -> [P, 1] -> scalar via matmul with ones
            lsum = small_pool.tile([P, 1], f32, tag="lsum")
            nc.vector.tensor_reduce(out=lsum[:], in_=loss_cols[:, :T],
                                    axis=mybir.AxisListType.X,
                                    op=mybir.AluOpType.add)
            ones = const_pool.tile([P, 1], f32)
            nc.vector.memset(ones[:], 1.0)
            lscalar = psum_pool.tile([1, 1], f32, space="PSUM", tag="lscalar")
            nc.tensor.matmul(out=lscalar[:], lhsT=lsum[:], rhs=ones[:],
                             start=True, stop=True)
            loss_sb = small_pool.tile([1, 1], f32, tag="loss_sb")
            nc.vector.tensor_copy(out=loss_sb[:], in_=lscalar[:])
            nc.sync.dma_start(loss_ap[:], loss_sb[:])

    nc.compile()
    return nc


def _make_in_maps(features, center_var, slot_feat_idx, slot_local, T):
    feat_padded = np.concatenate(
        [features, np.zeros((1, EMBED_DIM), np.float32)], axis=0)
    in_maps = []
    for c in range(N_CORES):
        shard_h = np.concatenate(
            [center_var[c * SHARD : (c + 1) * SHARD],
             np.zeros((1, EMBED_DIM), np.float32)], axis=0)
        # slot s = t*P + p  ->  SBUF layout [p, t]
        fi = slot_feat_idx[c].reshape(T, P)
        feat_h = np.ascontiguousarray(
            feat_padded[fi].transpose(1, 0, 2).reshape(P, T * EMBED_DIM))
        idx_h = np.ascontiguousarray(slot_local[c].reshape(T, P).T)
        in_maps.append({"shard": shard_h, "feat": feat_h, "idx": idx_h})
    return in_maps


def kernel(features, labels, center_var):
    from concourse.bass_utils import run_bass_kernel_spmd

    features = np.ascontiguousarray(np.asarray(features), dtype=np.float32)
    center_var = np.ascontiguousarray(np.asarray(center_var), dtype=np.float32)

    slot_feat_idx, slot_local, T = _route(labels)
    tile_deps = _tile_chunk_deps(slot_local, T)

    key = (T, tile_deps)
    if key not in _PROGRAM_CACHE:
        _PROGRAM_CACHE[key] = _build_program(T, tile_deps)
    nc = _PROGRAM_CACHE[key]

    in_maps = _make_in_maps(features, center_var, slot_feat_idx, slot_local, T)
    res = run_bass_kernel_spmd(nc, in_maps, list(range(N_CORES)))

    new_centers = np.concatenate(
        [res.results[c]["out"][:SHARD] for c in range(N_CORES)], axis=0)
    loss_sum = sum(float(res.results[c]["loss"][0, 0]) for c in range(N_CORES))
    loss = np.array(loss_sum / (BATCH * EMBED_DIM), dtype=np.float32)
    return loss, new_centers


# revision 16
# speedup vs baseline: 2.4048x; 1.0687x over previous
"""CenterLoss kernel for Trainium2 (8 NeuronCores, Bass/Tile).

Computes, for features [B, D], labels [B], center_var [V, D]:
    centers_batch = center_var[labels]
    loss          = mean((features - centers_batch)^2)
    new_centers   = center_var.at[labels].add(0.05 * (features - centers_batch))

Sharding: center_var is row-sharded across 8 cores (12500 rows each).
Each (feature, label) pair is routed on host to the core that owns its
label (the expert-parallel all-to-all), sorted by label, and bin-packed
into 128-slot tiles such that no label's duplicate run crosses a tile
boundary.  On device, each 128-pair tile gathers its center rows with an
indirect DMA, merges duplicate labels with an is_equal selection-matrix
matmul (PE accumulates duplicate updates; colliding scatter writes then
carry identical values), and scatters the updated rows back.  The bulk
shard copy is split into chunks; each scatter only waits for the copy
chunks covering its (sorted, hence contiguous) row band, so scatters
stream while the copy is still in flight.  The scalar loss is reduced
per core and summed on host.
"""

import numpy as np

NUM_CLASSES = 100000
EMBED_DIM = 512
BATCH = 16384
N_CORES = 8
SHARD = NUM_CLASSES // N_CORES  # 12500
P = 128
N_CHUNKS = 40
CHUNK = -(-SHARD // N_CHUNKS)  # 313; last chunk is smaller
_CHUNK_BOUNDS = [min(i * CHUNK, SHARD) for i in range(N_CHUNKS + 1)]

_PROGRAM_CACHE = {}


def _route(labels):
    """Route pairs to shard-owning cores; bin-pack into 128-slot tiles so a
    duplicate-label run never crosses a tile boundary.

    Returns (slot_feat_idx [N_CORES, T*P] int64 with -1 padding,
             slot_local    [N_CORES, T*P] int32 with SHARD padding, T).
    """
    labels = np.asarray(labels).astype(np.int64).reshape(-1)
    owner = labels // SHARD
    local = (labels % SHARD).astype(np.int32)

    per_core = []
    for c in range(N_CORES):
        sel = np.nonzero(owner == c)[0]
        loc = local[sel]
        order = np.argsort(loc, kind="stable")
        sel, loc = sel[order], loc[order]
        n = len(loc)
        if n == 0:
            per_core.append((np.empty(0, np.int64), np.empty(0, np.int32)))
            continue
        # run boundaries of equal labels
        change = np.nonzero(np.diff(loc))[0] + 1
        run_starts = np.concatenate(([0], change))
        run_lens = np.diff(np.concatenate((run_starts, [n])))
        if run_lens.max() > P:
            raise ValueError("a single label occurs more than 128 times")
        # greedy bin-pack: slot position for each run
        run_slots = np.empty(len(run_starts), np.int64)
        cur = 0
        for r, L in enumerate(run_lens):
            used = cur % P
            if used + L > P:
                cur += P - used
            run_slots[r] = cur
            cur += L
        pair_run = np.repeat(np.arange(len(run_starts)), run_lens)
        within = np.arange(n) - run_starts[pair_run]
        slots = run_slots[pair_run] + within
        total = int(cur)
        feat_idx = np.full(total, -1, np.int64)
        loc_out = np.full(total, SHARD, np.int32)
        feat_idx[slots] = sel
        loc_out[slots] = loc
        per_core.append((feat_idx, loc_out))

    max_slots = max(max(len(a) for a, _ in per_core), 1)
    T = -(-max_slots // P)
    n_slots = T * P
    slot_feat_idx = np.full((N_CORES, n_slots), -1, np.int64)
    slot_local = np.full((N_CORES, n_slots), SHARD, np.int32)
    for c, (fi, lo) in enumerate(per_core):
        slot_feat_idx[c, : len(fi)] = fi
        slot_local[c, : len(lo)] = lo
    return slot_feat_idx, slot_local, T


def _tile_chunk_deps(slot_local, T):
    """Per tile, the copy-chunk range [clo, chi] covering the union (over
    cores) of the tile's real local rows.  All-pad tiles get (0, -1)."""
    deps = []
    for t in range(T):
        rows = slot_local[:, t * P : (t + 1) * P]
        real = rows[rows < SHARD]
        if real.size == 0:
            deps.append((0, -1))
        else:
            deps.append((int(real.min()) // CHUNK, int(real.max()) // CHUNK))
    return tuple(deps)


def _build_program(T, tile_deps):
    """Build + compile the SPMD Bass program for T pair-tiles per core."""
    import concourse.bass as bass
    import concourse.tile as tile
    from concourse import bacc, mybir
    from concourse.masks import make_identity
    from concourse.tile_rust import add_dep_helper

    f32 = mybir.dt.float32
    i32 = mybir.dt.int32

    nc = bacc.Bacc("TRN2", target_bir_lowering=False, debug=False,
                   num_devices=N_CORES)

    shard_ap = nc.dram_tensor("shard", [SHARD + 1, EMBED_DIM], f32,
                              kind="ExternalInput").ap()
    feat_ap = nc.dram_tensor("feat", [P, T * EMBED_DIM], f32,
                             kind="ExternalInput").ap()
    idx_ap = nc.dram_tensor("idx", [P, T], i32, kind="ExternalInput").ap()
    out_ap = nc.dram_tensor("out", [SHARD + 1, EMBED_DIM], f32,
                            kind="ExternalOutput").ap()
    loss_ap = nc.dram_tensor("loss", [1, 1], f32, kind="ExternalOutput").ap()

    with tile.TileContext(nc) as tc:
        with (
            tc.tile_pool(name="const", bufs=1) as const_pool,
            tc.tile_pool(name="stage", bufs=1) as stage_pool,
            tc.tile_pool(name="work", bufs=4) as work_pool,
            tc.tile_pool(name="newp", bufs=max(1, min(T, 24))) as new_pool,
            tc.tile_pool(name="small", bufs=4) as small_pool,
            tc.tile_pool(name="psum", bufs=2, space="PSUM") as psum_pool,
            tc.tile_pool(name="psumT", bufs=2, space="PSUM") as psumT_pool,
        ):
            # stage routed features / indices FIRST: HWDGE queues are FIFO
            # per issuing engine, so these must not sit behind the bulk copy
            idx_sb = stage_pool.tile([P, T], i32)
            nc.sync.dma_start(idx_sb[:], idx_ap[:])
            # features fit SBUF as one staged block only for moderate T
            stage_feat = T <= 24
            if stage_feat:
                feat_sb = stage_pool.tile([P, T * EMBED_DIM], f32)
                nc.scalar.dma_start(feat_sb[:], feat_ap[:])

            # bulk copy of the shard rows input -> output in chunks,
            # alternating between the two HWDGE queues
            copy_insts = []
            for ci in range(N_CHUNKS):
                lo, hi = _CHUNK_BOUNDS[ci], _CHUNK_BOUNDS[ci + 1]
                eng = nc.sync if ci % 2 == 0 else nc.scalar
                copy_insts.append(
                    eng.dma_start(out=out_ap[lo:hi], in_=shard_ap[lo:hi]))

            identity = const_pool.tile([P, P], f32)
            make_identity(nc, identity[:])
            loss_cols = stage_pool.tile([P, T], f32)

            for t in range(T):
                idx_t = idx_sb[:, t : t + 1]
                if stage_feat:
                    feat_t = feat_sb[:, t * EMBED_DIM : (t + 1) * EMBED_DIM]
                else:
                    ftile = work_pool.tile([P, EMBED_DIM], f32, tag="ftile")
                    eng = nc.scalar if t % 2 == 0 else nc.sync
                    eng.dma_start(
                        ftile[:], feat_ap[:, t * EMBED_DIM : (t + 1) * EMBED_DIM])
                    feat_t = ftile[:]

                # gather original center rows for this tile's pairs
                c_t = work_pool.tile([P, EMBED_DIM], f32, tag="c")
                nc.gpsimd.indirect_dma_start(
                    out=c_t[:],
                    out_offset=None,
                    in_=shard_ap[:],
                    in_offset=bass.IndirectOffsetOnAxis(ap=idx_t, axis=0),
                )

                # selection matrix sel[i,j] = (label_i == label_j) within tile
                idxf = small_pool.tile([P, 1], f32, tag="idxf")
                nc.vector.tensor_copy(idxf[:], idx_t)
                pT = psumT_pool.tile([P, P], f32, space="PSUM", tag="pT")
                nc.tensor.transpose(out=pT[:], in_=idxf[:].to_broadcast([P, P]),
                                    identity=identity[:])
                sel = small_pool.tile([P, P], f32, tag="sel")
                nc.vector.tensor_tensor(
                    out=sel[:], in0=idxf[:].to_broadcast([P, P]), in1=pT[:],
                    op=mybir.AluOpType.is_equal)

                # e = f - c ; loss column = sum(e^2) fused into ACT square
                e_t = work_pool.tile([P, EMBED_DIM], f32, tag="e")
                nc.vector.tensor_tensor(
                    out=e_t[:], in0=feat_t, in1=c_t[:],
                    op=mybir.AluOpType.subtract)
                esq = work_pool.tile([P, EMBED_DIM], f32, tag="esq")
                nc.scalar.activation(
                    out=esq[:], in_=e_t[:],
                    func=mybir.ActivationFunctionType.Square,
                    accum_out=loss_cols[:, t : t + 1])

                # accumulate duplicate-label updates: acc = sel @ e
                acc = psum_pool.tile([P, EMBED_DIM], f32, space="PSUM", tag="acc")
                nc.tensor.matmul(out=acc[:], lhsT=sel[:], rhs=e_t[:],
                                 start=True, stop=True)

                # new rows = c + 0.05 * acc ; scatter back once the copy
                # chunks covering this tile's row band have landed
                new_t = new_pool.tile([P, EMBED_DIM], f32, tag="new")
                nc.vector.scalar_tensor_tensor(
                    out=new_t[:], in0=acc[:], scalar=0.05, in1=c_t[:],
                    op0=mybir.AluOpType.mult, op1=mybir.AluOpType.add)
                sc = nc.gpsimd.indirect_dma_start(
                    out=out_ap[:],
                    out_offset=bass.IndirectOffsetOnAxis(ap=idx_t, axis=0),
                    in_=new_t[:],
                    in_offset=None,
                    bounds_check=SHARD - 1,
                    oob_is_err=False,
                )
                clo, chi = tile_deps[t]
                for ci in range(clo, chi + 1):
                    add_dep_helper(sc.ins, copy_insts[ci].ins, sync=True,
                                   reason="scatter after its copy chunks")

            # reduce loss: [P, T] -> [P, 1] -> scalar via matmul with ones
            lsum = small_pool.tile([P, 1], f32, tag="lsum")
            nc.vector.tensor_reduce(out=lsum[:], in_=loss_cols[:, :T],
                                    axis=mybir.AxisListType.X,
                                    op=mybir.AluOpType.add)
            ones = const_pool.tile([P, 1], f32)
            nc.vector.memset(ones[:], 1.0)
            lscalar = psum_pool.tile([1, 1], f32, space="PSUM", tag="lscalar")
            nc.tensor.matmul(out=lscalar[:], lhsT=lsum[:], rhs=ones[:],
                             start=True, stop=True)
            loss_sb = small_pool.tile([1, 1], f32, tag="loss_sb")
            nc.vector.tensor_copy(out=loss_sb[:], in_=lscalar[:])
            nc.sync.dma_start(loss_ap[:], loss_sb[:])

    nc.compile()
    return nc


def _make_in_maps(features, center_var, slot_feat_idx, slot_local, T):
    feat_padded = np.concatenate(
        [features, np.zeros((1, EMBED_DIM), np.float32)], axis=0)
    in_maps = []
    for c in range(N_CORES):
        shard_h = np.concatenate(
            [center_var[c * SHARD : (c + 1) * SHARD],
             np.zeros((1, EMBED_DIM), np.float32)], axis=0)
        # slot s = t*P + p  ->  SBUF layout [p, t]
        fi = slot_feat_idx[c].reshape(T, P)
        feat_h = np.ascontiguousarray(
            feat_padded[fi].transpose(1, 0, 2).reshape(P, T * EMBED_DIM))
        idx_h = np.ascontiguousarray(slot_local[c].reshape(T, P).T)
        in_maps.append({"shard": shard_h, "feat": feat_h, "idx": idx_h})
    return in_maps


def kernel(features, labels, center_var):
    from concourse.bass_utils import run_bass_kernel_spmd

    features = np.ascontiguousarray(np.asarray(features), dtype=np.float32)
    center_var = np.ascontiguousarray(np.asarray(center_var), dtype=np.float32)

    slot_feat_idx, slot_local, T = _route(labels)
    tile_deps = _tile_chunk_deps(slot_local, T)

    key = (T, tile_deps)
    if key not in _PROGRAM_CACHE:
        _PROGRAM_CACHE[key] = _build_program(T, tile_deps)
    nc = _PROGRAM_CACHE[key]

    in_maps = _make_in_maps(features, center_var, slot_feat_idx, slot_local, T)
    res = run_bass_kernel_spmd(nc, in_maps, list(range(N_CORES)))

    new_centers = np.concatenate(
        [res.results[c]["out"][:SHARD] for c in range(N_CORES)], axis=0)
    loss_sum = sum(float(res.results[c]["loss"][0, 0]) for c in range(N_CORES))
    loss = np.array(loss_sum / (BATCH * EMBED_DIM), dtype=np.float32)
    return loss, new_centers
